# revision 1
# baseline (speedup 1.0000x reference)
"""CoSSM (bidirectional Mamba-style SSM) Trainium2 Bass kernel.

Sharding over 8 cores: (stream g/r) x (batch 0/1) x (d_inner half 0/1).
Each core, for its (stream, batch):
  - in_proj x (all 1536 ch, channel-permuted so own half = tiles 0..5)
  - causal conv (fwd) + anticausal conv (bwd) via diagonal PE matmuls, silu
  - xproj (full-d contraction, replicated within the pair) -> dt/B/C rows
  - dtproj + softplus (exp/ln) for its own 768 channels
  - fwd + bwd selective scans (16 modes, DVE tensor_tensor_scan)
  - D residual, bidirectional average (DMA time-flips), silu(z) gate
  - partial out_proj (contraction over its 768 channels) -> (2048, 768)
Host sums the two partial outputs per (stream, batch).

Self-contained: hardcodes shapes. Inputs use the reference setup_inputs names.
"""
import numpy as np

import concourse.bass as bass
import concourse.bacc as bacc
import concourse.tile as tile
import concourse.mybir as mybir
from concourse.bass_utils import run_bass_kernel_spmd

F32 = mybir.dt.float32
F32R = mybir.dt.float32r
AF = mybir.ActivationFunctionType
OP = mybir.AluOpType

D_MODEL = 768
D_STATE = 16
D_CONV = 4
D_INNER = 1536
DT_RANK = 48
NBATCH = 2
L = 2048
HALF = 768
NT_FULL = 12      # 128-tiles over d_inner
NT_HALF = 6       # 128-tiles over own half
NK = 6            # 128-chunks over d_model contraction
TCH = 512         # time chunk for projections
NTCH = L // TCH
SEG = 256         # time segment for the scan + epilogue
NSEG = L // SEG
PAD = 3

_PROGRAM_CACHE = {}


def _ap(t, offset, ap):
    return bass.AP(tensor=t.tensor, offset=offset, ap=[list(a) for a in ap])


def _bc6(tile2d):
    """[128, T] tile -> [128, 6, T] read view (step-0 middle dim)."""
    a = tile2d[:]
    return bass.AP(tensor=a.tensor, offset=a.offset,
                   ap=[list(a.ap[0]), [0, NT_HALF], list(a.ap[1])])


def build_program(a_vals_f, a_vals_b):
    nc = bacc.Bacc("TRN2", target_bir_lowering=False, debug=False, num_devices=8)

    def din(name, shape, dt):
        return nc.dram_tensor(name, list(shape), dt, kind="ExternalInput").ap()

    hid_T = din("hid_T", (D_MODEL, L), F32R)
    w_in_x_T = din("w_in_x_T", (D_MODEL, D_INNER), F32R)
    w_in_z_T = din("w_in_z_T", (D_MODEL, HALF), F32R)
    cdiag = {"f": din("cdiag_f", (NT_FULL * D_CONV, 128, 128), F32R),
             "b": din("cdiag_b", (NT_FULL * D_CONV, 128, 128), F32R)}
    cbias = {"f": din("cbias_f", (D_INNER,), F32),
             "b": din("cbias_b", (D_INNER,), F32)}
    w_x_T = {"f": din("w_x_T_f", (D_INNER, 80), F32R),
             "b": din("w_x_T_b", (D_INNER, 80), F32R)}
    w_dt_T = {"f": din("w_dt_T_f", (DT_RANK, HALF), F32),
              "b": din("w_dt_T_b", (DT_RANK, HALF), F32)}
    dt_bias = {"f": din("dt_bias_f", (HALF,), F32),
               "b": din("dt_bias_b", (HALF,), F32)}
    d_res = {"f": din("d_f", (HALF,), F32), "b": din("d_b", (HALF,), F32)}
    w_out_T = din("w_out_T", (HALF, D_MODEL), F32R)

    out = nc.dram_tensor("out", [L, D_MODEL], F32, kind="ExternalOutput").ap()

    x_sp = nc.dram_tensor("x_sp", [NT_FULL, 128, L + 6], F32R).ap()
    u_sp = {d: nc.dram_tensor(f"u_sp_{d}", [NT_HALF, 128, L], F32R).ap()
            for d in "fb"}
    dl_sp = {d: nc.dram_tensor(f"dl_sp_{d}", [NT_HALF, 128, L], F32).ap()
             for d in "fb"}
    r_sp = {d: nc.dram_tensor(f"r_sp_{d}", [NT_HALF, 128, L], F32).ap()
            for d in "fb"}
    xdbl_dram = {d: nc.dram_tensor(f"xdbl_{d}", [80, L], F32).ap() for d in "fb"}
    xdbl_scr = nc.dram_tensor("xdbl_scr", [80, L], F32).ap()

    a_vals = {"f": a_vals_f, "b": a_vals_b}

    import contextlib
    with tile.TileContext(nc) as tc, contextlib.ExitStack() as ctx:
        WPOOL = ctx.enter_context(tc.tile_pool(name="wsmall", bufs=1))
        HS = ctx.enter_context(tc.tile_pool(name="hidstream", bufs=1))
        WS = ctx.enter_context(tc.tile_pool(name="wstream", bufs=4))
        CD = ctx.enter_context(tc.tile_pool(name="cdiag", bufs=2))
        SP = ctx.enter_context(tc.tile_pool(name="work", bufs=2))
        UP = ctx.enter_context(tc.tile_pool(name="upool", bufs=3))
        SC = ctx.enter_context(tc.tile_pool(name="scan1", bufs=1))
        SC2 = ctx.enter_context(tc.tile_pool(name="scan2", bufs=2))
        PS = ctx.enter_context(tc.tile_pool(name="psum", bufs=2, space="PSUM"))
        PS1 = ctx.enter_context(tc.tile_pool(name="psum1", bufs=1, space="PSUM"))
        PER = ctx.enter_context(tc.tile_pool(name="persist", bufs=1))

        # ---- small resident weights ----
        def load_cols(src, n, tagn):
            t = WPOOL.tile([128, n], F32, tag=tagn)
            nc.sync.dma_start(out=t, in_=_ap(src, 0, [[1, 128], [128, n]]))
            return t

        t_cbias = {d: load_cols(cbias[d], NT_FULL, f"cb{d}") for d in "fb"}
        t_dtb = {d: load_cols(dt_bias[d], NT_HALF, f"db{d}") for d in "fb"}
        t_dcol = {d: load_cols(d_res[d], NT_HALF, f"dd{d}") for d in "fb"}
        t_wx = {}
        for d in "fb":
            t = WPOOL.tile([128, NT_FULL, 80], F32R, tag=f"wx{d}")
            nc.sync.dma_start(
                out=t, in_=_ap(w_x_T[d], 0,
                               [[80, 128], [128 * 80, NT_FULL], [1, 80]]))
            t_wx[d] = t
        t_wdt = {}
        for d in "fb":
            t = WPOOL.tile([128, HALF], F32, tag=f"wdt{d}")
            nc.sync.dma_start(out=t[0:DT_RANK, :], in_=w_dt_T[d])
            t_wdt[d] = t

        t_zero3 = WPOOL.tile([128, PAD], F32R, tag="zero3")
        nc.vector.memset(t_zero3[:].bitcast(F32), 0.0)

        # ---- phase 0: in_proj x -> x_sp (padded in DRAM) ----
        for i in range(NT_FULL):
            nc.sync.dma_start(out=_ap(x_sp, i * 128 * (L + 6),
                                      [[L + 6, 128], [1, PAD]]),
                              in_=t_zero3[:])
            nc.sync.dma_start(out=_ap(x_sp, i * 128 * (L + 6) + PAD + L,
                                      [[L + 6, 128], [1, PAD]]),
                              in_=t_zero3[:])
        for tci in range(NTCH):
            t0 = tci * TCH
            hid_tiles = []
            for k in range(NK):
                th = HS.tile([128, TCH], F32R, tag=f"hid{k}")
                nc.sync.dma_start(out=th, in_=hid_T[k * 128:(k + 1) * 128,
                                                    t0:t0 + TCH])
                hid_tiles.append(th)
            for i in range(NT_FULL):
                ps = PS.tile([128, TCH], F32, tag="mm512")
                for k in range(NK):
                    w = WS.tile([128, 128], F32R, tag="wxs")
                    nc.sync.dma_start(
                        out=w, in_=w_in_x_T[k * 128:(k + 1) * 128,
                                            i * 128:(i + 1) * 128])
                    nc.tensor.matmul(ps[:], w[:], hid_tiles[k][:],
                                     start=(k == 0), stop=(k == NK - 1))
                xo = SP.tile([128, TCH], F32R, tag="pre")
                nc.scalar.copy(xo[:], ps[:])
                nc.sync.dma_start(
                    out=_ap(x_sp, i * 128 * (L + 6) + PAD + t0,
                            [[L + 6, 128], [1, TCH]]),
                    in_=xo[:])

        # ---- per-direction pipeline ----
        for d in "fb":
            t_xdbl = PER.tile([128, L], F32, tag="xdbl_nat")
            for tci in range(NTCH):
                t0 = tci * TCH
                px = PS1.tile([128, TCH], F32, tag="xproj")
                for i in range(NT_FULL):
                    xs = SP.tile([128, TCH + 6], F32R, tag="xseg")
                    nc.sync.dma_start(
                        out=xs, in_=_ap(x_sp, i * 128 * (L + 6) + t0,
                                        [[L + 6, 128], [1, TCH + 6]]))
                    cp = PS.tile([128, TCH], F32, tag="mm512")
                    for k in range(D_CONV):
                        cd = CD.tile([128, 128], F32R, tag=f"cds{k}")
                        nc.sync.dma_start(out=cd, in_=cdiag[d][i * D_CONV + k])
                        off = k if d == "f" else 6 - k
                        nc.tensor.matmul(cp[:], cd[:], xs[:, off:off + TCH],
                                         start=(k == 0), stop=(k == D_CONV - 1))
                    pre = SP.tile([128, TCH], F32, tag="pre")
                    nc.scalar.activation(pre[:], cp[:], AF.Identity,
                                         bias=t_cbias[d][:, i:i + 1], scale=1.0)
                    s1 = SP.tile([128, TCH], F32, tag="s1")
                    nc.scalar.activation(s1[:], pre[:], AF.Exp, bias=0.0,
                                         scale=-1.0)
                    s2 = SP.tile([128, TCH], F32, tag="s2")
                    nc.scalar.activation(s2[:], s1[:], AF.Ln, bias=1.0, scale=1.0)
                    s3 = SP.tile([128, TCH], F32, tag="s3")
                    nc.scalar.activation(s3[:], s2[:], AF.Exp, bias=0.0,
                                         scale=-1.0)
                    u_i = UP.tile([128, TCH], F32R, tag="u_i")
                    nc.vector.tensor_tensor(u_i[:], pre[:], s3[:], OP.mult)
                    nc.tensor.matmul(px[0:80, :], t_wx[d][:, i, :], u_i[:],
                                     start=(i == 0), stop=(i == NT_FULL - 1))
                    if i < NT_HALF:
                        nc.sync.dma_start(
                            out=_ap(u_sp[d], i * 128 * L + t0,
                                    [[L, 128], [1, TCH]]),
                            in_=u_i[:])
                nc.scalar.copy(t_xdbl[0:80, t0:t0 + TCH], px[0:80, :])

            if d == "b":
                nc.sync.dma_start(out=xdbl_scr, in_=t_xdbl[0:80, :])
                t_xdbl2 = PER.tile([128, L], F32, tag="xdbl_flip")
                nc.sync.dma_start(
                    out=t_xdbl2[0:80, :],
                    in_=_ap(xdbl_scr, L - 1, [[L, 80], [-1, L]]))
                t_xdbl = t_xdbl2
            nc.sync.dma_start(out=xdbl_dram[d], in_=t_xdbl[0:80, :])

            for tci in range(NTCH):
                t0 = tci * TCH
                for m in range(NT_HALF):
                    dp = PS.tile([128, TCH], F32, tag="mm512")
                    nc.tensor.matmul(dp[:],
                                     t_wdt[d][0:DT_RANK, m * 128:(m + 1) * 128],
                                     t_xdbl[0:DT_RANK, t0:t0 + TCH],
                                     start=True, stop=True)
                    e1 = SP.tile([128, TCH], F32, tag="s1")
                    nc.scalar.activation(e1[:], dp[:], AF.Exp,
                                         bias=t_dtb[d][:, m:m + 1], scale=1.0)
                    dl = SP.tile([128, TCH], F32, tag="s2")
                    nc.scalar.activation(dl[:], e1[:], AF.Ln, bias=1.0, scale=1.0)
                    nc.sync.dma_start(
                        out=_ap(dl_sp[d], m * 128 * L + t0,
                                [[L, 128], [1, TCH]]),
                        in_=dl[:])

            # ---- scan ----
            carry = PER.tile([128, D_STATE, NT_HALF], F32, tag=f"carry{d}")
            for s in range(NSEG):
                t0 = s * SEG
                u_seg = SC.tile([128, NT_HALF, SEG], F32R, tag="u_seg")
                if d == "f":
                    nc.sync.dma_start(
                        out=u_seg,
                        in_=_ap(u_sp[d], t0,
                                [[L, 128], [128 * L, NT_HALF], [1, SEG]]))
                else:
                    for i in range(NT_HALF):
                        nc.sync.dma_start(
                            out=u_seg[:, i, :],
                            in_=_ap(u_sp[d], i * 128 * L + L - 1 - t0,
                                    [[L, 128], [-1, SEG]]))
                dl_seg = SC.tile([128, NT_HALF, SEG], F32, tag="dl_seg")
                nc.sync.dma_start(
                    out=dl_seg,
                    in_=_ap(dl_sp[d], t0,
                            [[L, 128], [128 * L, NT_HALF], [1, SEG]]))
                ud_seg = SC.tile([128, NT_HALF, SEG], F32, tag="ud_seg")
                nc.vector.tensor_tensor(ud_seg[:], dl_seg[:],
                                        u_seg[:].bitcast(F32), OP.mult)
                y_seg = SC.tile([128, NT_HALF, SEG], F32, tag="y_seg")
                for j in range(D_STATE):
                    brep = SC2.tile([128, SEG], F32, tag="brep")
                    nc.sync.dma_start(
                        out=brep, in_=_ap(xdbl_dram[d], (48 + j) * L + t0,
                                          [[0, 128], [1, SEG]]))
                    crep = SC2.tile([128, SEG], F32, tag="crep")
                    nc.sync.dma_start(
                        out=crep, in_=_ap(xdbl_dram[d], (64 + j) * L + t0,
                                          [[0, 128], [1, SEG]]))
                    a_j = SC2.tile([128, NT_HALF, SEG], F32, tag="a_j")
                    nc.scalar.activation(a_j[:], dl_seg[:], AF.Exp, bias=0.0,
                                         scale=float(a_vals[d][j]))
                    b_j = SC2.tile([128, NT_HALF, SEG], F32, tag="b_j")
                    nc.vector.tensor_tensor(b_j[:], ud_seg[:], _bc6(brep),
                                            OP.mult)
                    h_j = SC2.tile([128, NT_HALF, SEG], F32, tag="h_j")
                    for i in range(NT_HALF):
                        init = 0.0 if s == 0 else carry[:, j, i:i + 1]
                        nc.vector.tensor_tensor_scan(
                            h_j[:, i, :], a_j[:, i, :], b_j[:, i, :], init,
                            OP.mult, OP.add)
                    if s < NSEG - 1:
                        nc.vector.tensor_copy(
                            carry[:, j, :],
                            bass.AP(tensor=h_j.tensor,
                                    offset=h_j[:].offset + (SEG - 1),
                                    ap=[list(h_j[:].ap[0]), [SEG, NT_HALF]]))
                    tmp = SC2.tile([128, NT_HALF, SEG], F32, tag="tmp_j")
                    nc.vector.tensor_tensor(tmp[:], h_j[:], _bc6(crep), OP.mult)
                    if j == 0:
                        nc.gpsimd.tensor_copy(out=y_seg[:], in_=tmp[:])
                    elif j % 2 == 1:
                        nc.vector.tensor_tensor(y_seg[:], y_seg[:], tmp[:],
                                                OP.add)
                    else:
                        nc.gpsimd.tensor_tensor(y_seg[:], y_seg[:], tmp[:],
                                                OP.add)
                r_seg = SC.tile([128, NT_HALF, SEG], F32, tag="r_seg")
                for i in range(NT_HALF):
                    nc.vector.scalar_tensor_tensor(
                        r_seg[:, i, :], u_seg[:, i, :].bitcast(F32),
                        t_dcol[d][:, i:i + 1], y_seg[:, i, :], OP.mult, OP.add)
                nc.sync.dma_start(
                    out=_ap(r_sp[d], t0,
                            [[L, 128], [128 * L, NT_HALF], [1, SEG]]),
                    in_=r_seg[:])

        # ---- phase D: z-gate + combine + out_proj (SEG chunks) ----
        for s in range(NSEG):
            t0 = s * SEG
            hidz = []
            for k in range(NK):
                th = HS.tile([128, SEG], F32R, tag=f"hid{k}")
                nc.sync.dma_start(out=th, in_=hid_T[k * 128:(k + 1) * 128,
                                                    t0:t0 + SEG])
                hidz.append(th)
            gate = SC.tile([128, NT_HALF, SEG], F32, tag="y_seg")
            for m in range(NT_HALF):
                zp_ps = PS.tile([128, SEG], F32, tag="mm512")
                for k in range(NK):
                    w = WS.tile([128, 128], F32R, tag="wxs")
                    nc.sync.dma_start(
                        out=w, in_=w_in_z_T[k * 128:(k + 1) * 128,
                                            m * 128:(m + 1) * 128])
                    nc.tensor.matmul(zp_ps[:], w[:], hidz[k][:],
                                     start=(k == 0), stop=(k == NK - 1))
                zp = SP.tile([128, SEG], F32, tag="pre")
                nc.scalar.activation(zp[:], zp_ps[:], AF.Identity, bias=0.0,
                                     scale=1.0)
                zs1 = SP.tile([128, SEG], F32, tag="s1")
                nc.scalar.activation(zs1[:], zp_ps[:], AF.Exp, bias=0.0,
                                     scale=-1.0)
                zs2 = SP.tile([128, SEG], F32, tag="s2")
                nc.scalar.activation(zs2[:], zs1[:], AF.Ln, bias=1.0, scale=1.0)
                zs3 = SP.tile([128, SEG], F32, tag="s3")
                nc.scalar.activation(zs3[:], zs2[:], AF.Exp, bias=0.0,
                                     scale=-1.0)
                nc.vector.scalar_tensor_tensor(
                    gate[:, m, :], zp[:], 0.5, zs3[:], OP.mult, OP.mult)
            rf = SC.tile([128, NT_HALF, SEG], F32, tag="u_seg")
            nc.sync.dma_start(
                out=rf, in_=_ap(r_sp["f"], t0,
                                [[L, 128], [128 * L, NT_HALF], [1, SEG]]))
            rb = SC.tile([128, NT_HALF, SEG], F32, tag="dl_seg")
            for i in range(NT_HALF):
                nc.sync.dma_start(
                    out=rb[:, i, :],
                    in_=_ap(r_sp["b"], i * 128 * L + L - 1 - t0,
                            [[L, 128], [-1, SEG]]))
            comb = SC.tile([128, NT_HALF, SEG], F32, tag="ud_seg")
            nc.vector.tensor_tensor(comb[:], rf[:], rb[:], OP.add)
            yg = SC.tile([128, NT_HALF, SEG], F32R, tag="r_seg")
            nc.vector.tensor_tensor(yg[:], comb[:], gate[:], OP.mult)
            for tcc in range(SEG // 128):
                oseg = SP.tile([128, D_MODEL], F32, tag="oseg")
                for nh in range(2):
                    po = PS.tile([128, 384], F32, tag="oproj")
                    for i in range(NT_HALF):
                        wo = WS.tile([128, 384], F32R, tag="wo")
                        nc.sync.dma_start(
                            out=wo, in_=w_out_T[i * 128:(i + 1) * 128,
                                               nh * 384:(nh + 1) * 384])
                        nc.tensor.matmul(
                            po[:], yg[:, i, tcc * 128:(tcc + 1) * 128], wo[:],
                            start=(i == 0), stop=(i == NT_HALF - 1))
                    nc.scalar.copy(oseg[:, nh * 384:(nh + 1) * 384], po[:])
                nc.sync.dma_start(
                    out=out[t0 + tcc * 128:t0 + (tcc + 1) * 128, :],
                    in_=oseg[:])

    nc.compile()
    return nc


def _diags(w):  # (1536, 4) -> (48, 128, 128) diag tiles
    o = np.zeros((NT_FULL * D_CONV, 128, 128), np.float32)
    for i in range(NT_FULL):
        for k in range(D_CONV):
            np.fill_diagonal(o[i * D_CONV + k], w[i * 128:(i + 1) * 128, k])
    return o


def _prep_core_inputs(stream, b_idx, half, inp):
    p = "g" if stream == 0 else "r"
    h0, h1 = half * HALF, (half + 1) * HALF
    perm = np.r_[h0:h1, 0:h0, h1:D_INNER]  # own half first

    hs = np.asarray(inp[f"{p}_hidden_states"])[b_idx]
    w_in = np.asarray(inp[f"{p}_in_proj_w"])
    m = {
        "hid_T": np.ascontiguousarray(hs.T, dtype=np.float32),
        "w_in_x_T": np.ascontiguousarray(
            w_in[:D_INNER, :][perm].T, dtype=np.float32),
        "w_in_z_T": np.ascontiguousarray(
            w_in[D_INNER + h0:D_INNER + h1, :].T, dtype=np.float32),
        "cdiag_f": _diags(np.asarray(inp[f"{p}_conv_w"])[:, 0, :][perm]),
        "cdiag_b": _diags(np.asarray(inp[f"{p}_conv_w_bwd"])[:, 0, :][perm]),
        "cbias_f": np.ascontiguousarray(
            np.asarray(inp[f"{p}_conv_bias"])[perm], dtype=np.float32),
        "cbias_b": np.ascontiguousarray(
            np.asarray(inp[f"{p}_conv_bias_bwd"])[perm], dtype=np.float32),
        "w_x_T_f": np.ascontiguousarray(
            np.asarray(inp[f"{p}_xproj_w"])[:, perm].T, dtype=np.float32),
        "w_x_T_b": np.ascontiguousarray(
            np.asarray(inp[f"{p}_xproj_w_bwd"])[:, perm].T, dtype=np.float32),
        "w_dt_T_f": np.ascontiguousarray(
            np.asarray(inp[f"{p}_dtproj_w"])[h0:h1, :].T, dtype=np.float32),
        "w_dt_T_b": np.ascontiguousarray(
            np.asarray(inp[f"{p}_dtproj_w_bwd"])[h0:h1, :].T, dtype=np.float32),
        "dt_bias_f": np.ascontiguousarray(
            np.asarray(inp[f"{p}_dtproj_bias"])[h0:h1], dtype=np.float32),
        "dt_bias_b": np.ascontiguousarray(
            np.asarray(inp[f"{p}_dtproj_bias_bwd"])[h0:h1], dtype=np.float32),
        "d_f": np.ascontiguousarray(
            np.asarray(inp[f"{p}_D"])[h0:h1], dtype=np.float32),
        "d_b": np.ascontiguousarray(
            np.asarray(inp[f"{p}_D_bwd"])[h0:h1], dtype=np.float32),
        "w_out_T": np.ascontiguousarray(
            np.asarray(inp[f"{p}_out_w"])[:, h0:h1].T, dtype=np.float32),
    }
    return m


def kernel(**inputs):
    A_log = np.asarray(inputs["A_log"])
    A_log_b = np.asarray(inputs["A_log_bwd"])
    assert np.allclose(A_log, A_log[0:1, :]), "A_log must be d-independent"
    assert np.allclose(A_log_b, A_log_b[0:1, :]), "A_log_bwd must be d-independent"
    A_f = -np.exp(A_log[0].astype(np.float64))
    A_b = -np.exp(A_log_b[0].astype(np.float64))

    key = (tuple(np.round(A_f, 10)), tuple(np.round(A_b, 10)))
    if key not in _PROGRAM_CACHE:
        _PROGRAM_CACHE[key] = build_program(list(A_f), list(A_b))
    nc = _PROGRAM_CACHE[key]

    in_maps = []
    for stream in range(2):
        for b_idx in range(NBATCH):
            for half in range(2):
                in_maps.append(_prep_core_inputs(stream, b_idx, half, inputs))

    res = run_bass_kernel_spmd(nc, in_maps, list(range(8)))
    outs = [r["out"] for r in res.results]

    g_out = np.stack([outs[0] + outs[1], outs[2] + outs[3]])
    r_out = np.stack([outs[4] + outs[5], outs[6] + outs[7]])
    return (np.asarray(g_out, np.float32), np.asarray(r_out, np.float32))



# revision 23
# speedup vs baseline: 2.8272x; 2.8272x over previous
"""CoSSM (bidirectional Mamba-style SSM) Trainium2 Bass kernel.

Sharding over 8 cores: (stream g/r) x (batch 0/1) x (d_inner half 0/1).
Each core, for its (stream, batch):
  - in_proj x (all 1536 ch, channel-permuted so own half = tiles 0..5),
    recomputed per direction, fused with the causal/anticausal depthwise
    conv (diagonal PE matmuls) + native Silu, chunked over time with a
    4-column halo carried between chunks (no DRAM round trip for x)
  - xproj (full-d contraction) -> dt/B/C rows; dtproj + native Softplus
  - fwd + bwd selective scans (16 states, DVE tensor_tensor_scan);
    bwd consumes u/dl/B/C through time-reversed DMA reads (no flips)
  - D residual, bidirectional average (0.5 folded into out_proj w),
    silu(z) gate, partial out_proj -> (2048, 768)
Host sums the two partial outputs per (stream, batch).

Weights are SBUF-resident (loaded once, bf16); activations bf16 where
cheap, f32 on the delta/scan-state path.

Self-contained: hardcodes shapes. Inputs use the reference setup_inputs names.
"""
import numpy as np
import ml_dtypes

import concourse.bass as bass
import concourse.bacc as bacc
import concourse.tile as tile
import concourse.mybir as mybir
from concourse.bass_utils import run_bass_kernel_spmd

F32 = mybir.dt.float32
BF16 = mybir.dt.bfloat16
AF = mybir.ActivationFunctionType
OP = mybir.AluOpType

D_MODEL = 768
D_STATE = 16
D_CONV = 4
D_INNER = 1536
DT_RANK = 48
NBATCH = 2
L = 2048
HALF = 768
NT_FULL = 12      # 128-tiles over d_inner
NT_HALF = 6       # 128-tiles over own half
NK = 6            # 128-chunks over d_model contraction
TCH = 512         # time chunk for phase A
NTCH = L // TCH
SEG = 256         # time segment for the scan + epilogue
NSEG = L // SEG
HALO = 4

_PROGRAM_CACHE = {}


def _ap(t, offset, ap):
    return bass.AP(tensor=t.tensor, offset=offset, ap=[list(a) for a in ap])


def _bc6(view2d):
    """[128, T] AP -> [128, 6, T] read view (0-stride middle dim)."""
    a = view2d
    return bass.AP(tensor=a.tensor, offset=a.offset,
                   ap=[list(a.ap[0]), [0, NT_HALF], list(a.ap[1])])


def build_program(a_vals_f, a_vals_b):
    nc = bacc.Bacc("TRN2", target_bir_lowering=False, debug=False, num_devices=8)

    def din(name, shape, dt):
        return nc.dram_tensor(name, list(shape), dt, kind="ExternalInput").ap()

    hid_T = din("hid_T", (D_MODEL, L), BF16)
    w_in_x_T = din("w_in_x_T", (D_MODEL, D_INNER), BF16)
    w_zo_T = din("w_zo_T", (D_MODEL, 2 * D_MODEL), BF16)  # [w_in_z | 0.5*w_out]
    cdiag = {"f": din("cdiag_f", (NT_FULL * D_CONV, 128, 128), BF16),
             "b": din("cdiag_b", (NT_FULL * D_CONV, 128, 128), BF16)}
    cbias = {"f": din("cbias_f", (D_INNER,), F32),
             "b": din("cbias_b", (D_INNER,), F32)}
    w_x_T = {"f": din("w_x_T_f", (D_INNER, 80), BF16),
             "b": din("w_x_T_b", (D_INNER, 80), BF16)}
    w_dt_T = {"f": din("w_dt_T_f", (DT_RANK, HALF), F32),
              "b": din("w_dt_T_b", (DT_RANK, HALF), F32)}
    dt_bias = {"f": din("dt_bias_f", (HALF,), F32),
               "b": din("dt_bias_b", (HALF,), F32)}
    d_res = {"f": din("d_f", (HALF,), F32), "b": din("d_b", (HALF,), F32)}

    out = nc.dram_tensor("out", [L, D_MODEL], F32, kind="ExternalOutput").ap()

    u_sp = {d: nc.dram_tensor(f"u_sp_{d}", [NT_HALF, 128, L], BF16).ap()
            for d in "fb"}
    dl_sp = {d: nc.dram_tensor(f"dl_sp_{d}", [NT_HALF, 128, L], F32).ap()
             for d in "fb"}
    r_sp = {d: nc.dram_tensor(f"r_sp_{d}", [NT_HALF, 128, L], F32).ap()
            for d in "fb"}
    bc_dram = {d: nc.dram_tensor(f"bc_{d}", [2 * D_STATE, L], BF16).ap()
               for d in "fb"}

    a_vals = {"f": a_vals_f, "b": a_vals_b}

    import contextlib
    with tile.TileContext(nc) as tc, contextlib.ExitStack() as ctx:
        WPOOL = ctx.enter_context(tc.tile_pool(name="wsmall", bufs=1))
        AP_ = ctx.enter_context(tc.tile_pool(name="phaseA", bufs=1))
        SP = ctx.enter_context(tc.tile_pool(name="work", bufs=2))
        SC = ctx.enter_context(tc.tile_pool(name="scan1", bufs=1))
        SC2 = ctx.enter_context(tc.tile_pool(name="scan2", bufs=2))
        DP = ctx.enter_context(tc.tile_pool(name="phaseD", bufs=1))
        PS = ctx.enter_context(tc.tile_pool(name="psum", bufs=2, space="PSUM"))
        PS1 = ctx.enter_context(tc.tile_pool(name="psum1", bufs=1, space="PSUM"))
        PER = ctx.enter_context(tc.tile_pool(name="persist", bufs=1))

        # ---- resident weights (one DMA each) ----
        t_wbig = WPOOL.tile([128, NK, D_INNER], BF16, tag="wbig")
        nc.sync.dma_start(
            out=t_wbig,
            in_=_ap(w_in_x_T, 0,
                    [[D_INNER, 128], [128 * D_INNER, NK], [1, D_INNER]]))

        def load_cols(src, n, tagn):
            t = WPOOL.tile([128, n], F32, tag=tagn)
            nc.sync.dma_start(out=t, in_=_ap(src, 0, [[1, 128], [128, n]]))
            return t

        t_cbias = {d: load_cols(cbias[d], NT_FULL, f"cb{d}") for d in "fb"}
        t_dtb = {d: load_cols(dt_bias[d], NT_HALF, f"db{d}") for d in "fb"}
        t_dcol = {d: load_cols(d_res[d], NT_HALF, f"dd{d}") for d in "fb"}
        t_wx = {}
        for d in "fb":
            t = WPOOL.tile([128, NT_FULL, 80], BF16, tag=f"wx{d}")
            nc.sync.dma_start(
                out=t, in_=_ap(w_x_T[d], 0,
                               [[80, 128], [128 * 80, NT_FULL], [1, 80]]))
            t_wx[d] = t
        t_wdt = {}
        for d in "fb":
            t = WPOOL.tile([128, HALF], F32, tag=f"wdt{d}")
            nc.sync.dma_start(out=t[0:DT_RANK, :], in_=w_dt_T[d])
            t_wdt[d] = t

        # conv diag tiles: loaded per direction into one aliased slot
        def load_cdiag(d):
            t = WPOOL.tile([128, NT_FULL * D_CONV, 128], BF16, tag="cdiag")
            nc.sync.dma_start(
                out=t, in_=_ap(cdiag[d], 0,
                               [[128, 128], [128 * 128, NT_FULL * D_CONV],
                                [1, 128]]))
            return t

        t_xdbl = PER.tile([128, L], F32, tag="xdbl")
        t_xw = PER.tile([128, NT_FULL, TCH + HALO], BF16, tag="xw")
        carry = {d: PER.tile([128, D_STATE, NT_HALF], F32, tag=f"carry{d}",
                             name=f"carry_{d}")
                 for d in "fb"}

        # ---- phase A: fused in_proj + conv + silu + xproj + dt ----
        def emit_phaseA_chunk(d, tci, t_cd):
            t0 = tci * TCH
            first = (tci == 0) if d == "f" else (tci == NTCH - 1)
            th = AP_.tile([128, NK, TCH], BF16, tag="hidw", bufs=2)
            nc.sync.dma_start(
                out=th, in_=_ap(hid_T, t0, [[L, 128], [128 * L, NK], [1, TCH]]))
            # halo carry in xw: fwd cols 0:4 <- prev cols 512:516 (or zero);
            # bwd cols 512:516 <- prev cols 0:4 (or zero)
            if d == "f":
                if first:
                    nc.vector.memset(t_xw[:, :, 0:HALO].bitcast(F32), 0.0)
                else:
                    nc.vector.tensor_copy(t_xw[:, :, 0:HALO],
                                          t_xw[:, :, TCH:TCH + HALO])
            else:
                if first:
                    nc.vector.memset(t_xw[:, :, TCH:].bitcast(F32), 0.0)
                else:
                    nc.vector.tensor_copy(t_xw[:, :, TCH:TCH + HALO],
                                          t_xw[:, :, 0:HALO])
            xcol = HALO if d == "f" else 0

            t_useg = AP_.tile([128, NT_HALF, TCH], BF16, tag="useg")
            t_dlseg = AP_.tile([128, NT_HALF, TCH], F32, tag="dlseg")
            px = PS1.tile([128, TCH], F32, tag="xproj")
            for i in range(NT_FULL):
                ip = PS.tile([128, TCH], F32, tag="mm512")
                for k in range(NK):
                    nc.tensor.matmul(ip[:], t_wbig[:, k, i * 128:(i + 1) * 128],
                                     th[:, k, :],
                                     start=(k == 0), stop=(k == NK - 1))
                nc.scalar.copy(t_xw[:, i, xcol:xcol + TCH], ip[:])
                cp = PS.tile([128, TCH], F32, tag="mm512")
                for k in range(D_CONV):
                    off = 1 + k if d == "f" else 3 - k
                    nc.tensor.matmul(cp[:], t_cd[:, i * D_CONV + k, :],
                                     t_xw[:, i, off:off + TCH],
                                     start=(k == 0), stop=(k == D_CONV - 1))
                if i < NT_HALF:
                    u_i = t_useg[:, i, :]
                else:
                    u_hi = SP.tile([128, TCH], BF16, tag="u_hi")
                    u_i = u_hi[:]
                nc.scalar.activation(u_i, cp[:], AF.Silu,
                                     bias=t_cbias[d][:, i:i + 1], scale=1.0)
                nc.tensor.matmul(px[0:80, :], t_wx[d][:, i, :], u_i,
                                 start=(i == 0), stop=(i == NT_FULL - 1))
            nc.sync.dma_start(
                out=_ap(u_sp[d], t0, [[L, 128], [128 * L, NT_HALF], [1, TCH]]),
                in_=t_useg[:])
            nc.scalar.copy(t_xdbl[0:80, t0:t0 + TCH], px[0:80, :])
            for m in range(NT_HALF):
                dp = PS.tile([128, TCH], F32, tag="mm512")
                nc.tensor.matmul(dp[:],
                                 t_wdt[d][0:DT_RANK, m * 128:(m + 1) * 128],
                                 t_xdbl[0:DT_RANK, t0:t0 + TCH],
                                 start=True, stop=True)
                e1 = SP.tile([128, TCH], F32, tag="e1")
                nc.scalar.activation(e1[:], dp[:], AF.Exp,
                                     bias=t_dtb[d][:, m:m + 1], scale=1.0)
                nc.scalar.activation(t_dlseg[:, m, :], e1[:], AF.Ln,
                                     bias=1.0, scale=1.0)
            nc.sync.dma_start(
                out=_ap(dl_sp[d], t0, [[L, 128], [128 * L, NT_HALF], [1, TCH]]),
                in_=t_dlseg[:])

        # ---- scan ----
        def emit_scan_seg(d, s):
            # Scan seg s covers natural times [base, base+SEG). For d == "b"
            # all data stays in natural time order; only the scan instruction
            # operands use reversed views so the recurrence runs backward.
            rev = (d == "b")
            base = (L - (s + 1) * SEG) if rev else s * SEG
            u_seg = SC.tile([128, NT_HALF, SEG], BF16, tag="u_seg", bufs=2)
            nc.sync.dma_start(
                out=u_seg,
                in_=_ap(u_sp[d], base,
                        [[L, 128], [128 * L, NT_HALF], [1, SEG]]))
            dl_seg = SC.tile([128, NT_HALF, SEG], F32, tag="dl_seg")
            nc.sync.dma_start(
                out=dl_seg,
                in_=_ap(dl_sp[d], base,
                        [[L, 128], [128 * L, NT_HALF], [1, SEG]]))
            bc = SC.tile([128, 2 * D_STATE, SEG], BF16, tag="bc")
            nc.sync.dma_start(
                out=bc,
                in_=_ap(bc_dram[d], base,
                        [[0, 128], [L, 2 * D_STATE], [1, SEG]]))
            ud_seg = SC.tile([128, NT_HALF, SEG], F32, tag="ud_seg")
            nc.vector.tensor_tensor(ud_seg[:], dl_seg[:], u_seg[:], OP.mult)
            y_seg = SC.tile([128, NT_HALF, SEG], F32, tag="y_seg")
            for j in range(D_STATE):
                a_j = SC2.tile([128, NT_HALF, SEG], F32, tag="a_j")
                nc.scalar.activation(a_j[:], dl_seg[:], AF.Exp, bias=0.0,
                                     scale=float(a_vals[d][j]))
                b_j = SC2.tile([128, NT_HALF, SEG], F32, tag="b_j")
                nc.vector.tensor_tensor(b_j[:], ud_seg[:], _bc6(bc[:, j, :]),
                                        OP.mult)
                h_j = SC2.tile([128, NT_HALF, SEG], F32, tag="h_j")

                def sop(t, i):
                    v = t[:, i, :]
                    if not rev:
                        return v
                    return bass.AP(tensor=v.tensor, offset=v.offset + SEG - 1,
                                   ap=[list(v.ap[0]), [-1, SEG]])

                for i in range(NT_HALF):
                    init = 0.0 if s == 0 else carry[d][:, j, i:i + 1]
                    nc.vector.tensor_tensor_scan(
                        sop(h_j, i), sop(a_j, i), sop(b_j, i), init,
                        OP.mult, OP.add)
                if s < NSEG - 1:
                    # scan-order last element: natural col SEG-1 (fwd), 0 (bwd)
                    nc.vector.tensor_copy(
                        carry[d][:, j, :],
                        bass.AP(tensor=h_j.tensor,
                                offset=h_j[:].offset +
                                (0 if rev else SEG - 1),
                                ap=[list(h_j[:].ap[0]), [SEG, NT_HALF]]))
                nc.vector.tensor_tensor(h_j[:], h_j[:],
                                        _bc6(bc[:, D_STATE + j, :]), OP.mult)
                if j == 0:
                    nc.gpsimd.tensor_copy(out=y_seg[:], in_=h_j[:])
                elif j % 2 == 1:
                    nc.vector.tensor_tensor(y_seg[:], y_seg[:], h_j[:], OP.add)
                else:
                    nc.gpsimd.tensor_tensor(y_seg[:], y_seg[:], h_j[:], OP.add)
            r_seg = SC.tile([128, NT_HALF, SEG], F32, tag="r_seg")
            for i in range(NT_HALF):
                nc.vector.scalar_tensor_tensor(
                    r_seg[:, i, :], u_seg[:, i, :],
                    t_dcol[d][:, i:i + 1], y_seg[:, i, :], OP.mult, OP.add)
            nc.sync.dma_start(
                out=_ap(r_sp[d], base,
                        [[L, 128], [128 * L, NT_HALF], [1, SEG]]),
                in_=r_seg[:])

        # ---- phase D: z-gate + combine + out_proj ----
        def emit_D_seg(s):
            t0 = s * SEG
            th = AP_.tile([128, NK, SEG], BF16, tag="hidw", bufs=2)
            nc.sync.dma_start(
                out=th, in_=_ap(hid_T, t0, [[L, 128], [128 * L, NK], [1, SEG]]))
            gate = DP.tile([128, NT_HALF, SEG], BF16, tag="gate")
            for m in range(NT_HALF):
                zp = PS.tile([128, SEG], F32, tag="mm512")
                for k in range(NK):
                    nc.tensor.matmul(zp[:], t_wbig[:, k, m * 128:(m + 1) * 128],
                                     th[:, k, :],
                                     start=(k == 0), stop=(k == NK - 1))
                nc.scalar.activation(gate[:, m, :], zp[:], AF.Silu, bias=0.0,
                                     scale=1.0)
            rf = DP.tile([128, NT_HALF, SEG], F32, tag="rf")
            nc.sync.dma_start(
                out=rf, in_=_ap(r_sp["f"], t0,
                                [[L, 128], [128 * L, NT_HALF], [1, SEG]]))
            rb = DP.tile([128, NT_HALF, SEG], F32, tag="rb")
            nc.sync.dma_start(
                out=rb, in_=_ap(r_sp["b"], t0,
                                [[L, 128], [128 * L, NT_HALF], [1, SEG]]))
            nc.vector.tensor_tensor(rf[:], rf[:], rb[:], OP.add)
            yg = DP.tile([128, NT_HALF, SEG], BF16, tag="yg")
            nc.vector.tensor_tensor(yg[:], rf[:], gate[:], OP.mult)
            for tcc in range(SEG // 128):
                oseg = SP.tile([128, D_MODEL], F32, tag="oseg")
                for nh in range(2):
                    po = PS.tile([128, 384], F32, tag="oproj")
                    for i in range(NT_HALF):
                        nc.tensor.matmul(
                            po[:], yg[:, i, tcc * 128:(tcc + 1) * 128],
                            t_wbig[:, i, D_MODEL + nh * 384:
                                   D_MODEL + (nh + 1) * 384],
                            start=(i == 0), stop=(i == NT_HALF - 1))
                    nc.scalar.copy(oseg[:, nh * 384:(nh + 1) * 384], po[:])
                nc.sync.dma_start(
                    out=out[t0 + tcc * 128:t0 + (tcc + 1) * 128, :],
                    in_=oseg[:])

        # ---- sequencing: overlap scan(f) with A(b), scan(b) with D ----
        t_cd = load_cdiag("f")
        for tci in range(NTCH):
            emit_phaseA_chunk("f", tci, t_cd)
        nc.gpsimd.dma_start(out=bc_dram["f"],
                            in_=t_xdbl[DT_RANK:DT_RANK + 2 * D_STATE, :])
        t_cd = load_cdiag("b")
        for s in range(NSEG):
            emit_scan_seg("f", s)
            if s % 2 == 1:
                emit_phaseA_chunk("b", NTCH - 1 - s // 2, t_cd)
        nc.gpsimd.dma_start(out=bc_dram["b"],
                            in_=t_xdbl[DT_RANK:DT_RANK + 2 * D_STATE, :])
        # reload wbig slot with [w_in_z | 0.5*w_out] for phase D
        nc.sync.dma_start(
            out=t_wbig,
            in_=_ap(w_zo_T, 0,
                    [[2 * D_MODEL, 128], [128 * 2 * D_MODEL, NK],
                     [1, 2 * D_MODEL]]))
        for s in range(NSEG):
            emit_scan_seg("b", s)
            emit_D_seg(NSEG - 1 - s)

    nc.compile()
    return nc


def _diags(w):  # (1536, 4) -> (48, 128, 128) diag tiles
    o = np.zeros((NT_FULL * D_CONV, 128, 128), np.float32)
    for i in range(NT_FULL):
        for k in range(D_CONV):
            np.fill_diagonal(o[i * D_CONV + k], w[i * 128:(i + 1) * 128, k])
    return o


def _bf(a):
    return np.ascontiguousarray(np.asarray(a, np.float32).astype(
        ml_dtypes.bfloat16))


def _f32(a):
    return np.ascontiguousarray(np.asarray(a), dtype=np.float32)


def _prep_core_inputs(stream, b_idx, half, inp):
    p = "g" if stream == 0 else "r"
    h0, h1 = half * HALF, (half + 1) * HALF
    perm = np.r_[h0:h1, 0:h0, h1:D_INNER]  # own half first

    hs = np.asarray(inp[f"{p}_hidden_states"])[b_idx]
    w_in = np.asarray(inp[f"{p}_in_proj_w"])
    w_zo = np.concatenate(
        [np.asarray(w_in[D_INNER + h0:D_INNER + h1, :]).T,
         0.5 * np.asarray(inp[f"{p}_out_w"])[:, h0:h1].T], axis=1)
    m = {
        "hid_T": _bf(hs.T),
        "w_in_x_T": _bf(w_in[:D_INNER, :][perm].T),
        "w_zo_T": _bf(w_zo),
        "cdiag_f": _bf(_diags(np.asarray(inp[f"{p}_conv_w"])[:, 0, :][perm])),
        "cdiag_b": _bf(_diags(np.asarray(inp[f"{p}_conv_w_bwd"])[:, 0, :][perm])),
        "cbias_f": _f32(np.asarray(inp[f"{p}_conv_bias"])[perm]),
        "cbias_b": _f32(np.asarray(inp[f"{p}_conv_bias_bwd"])[perm]),
        "w_x_T_f": _bf(np.asarray(inp[f"{p}_xproj_w"])[:, perm].T),
        "w_x_T_b": _bf(np.asarray(inp[f"{p}_xproj_w_bwd"])[:, perm].T),
        "w_dt_T_f": _f32(np.asarray(inp[f"{p}_dtproj_w"])[h0:h1, :].T),
        "w_dt_T_b": _f32(np.asarray(inp[f"{p}_dtproj_w_bwd"])[h0:h1, :].T),
        "dt_bias_f": _f32(np.asarray(inp[f"{p}_dtproj_bias"])[h0:h1]),
        "dt_bias_b": _f32(np.asarray(inp[f"{p}_dtproj_bias_bwd"])[h0:h1]),
        "d_f": _f32(np.asarray(inp[f"{p}_D"])[h0:h1]),
        "d_b": _f32(np.asarray(inp[f"{p}_D_bwd"])[h0:h1]),
    }
    return m


def kernel(**inputs):
    A_log = np.asarray(inputs["A_log"])
    A_log_b = np.asarray(inputs["A_log_bwd"])
    assert np.allclose(A_log, A_log[0:1, :]), "A_log must be d-independent"
    assert np.allclose(A_log_b, A_log_b[0:1, :]), "A_log_bwd must be d-independent"
    A_f = -np.exp(A_log[0].astype(np.float64))
    A_b = -np.exp(A_log_b[0].astype(np.float64))

    key = (tuple(np.round(A_f, 10)), tuple(np.round(A_b, 10)))
    if key not in _PROGRAM_CACHE:
        _PROGRAM_CACHE[key] = build_program(list(A_f), list(A_b))
    nc = _PROGRAM_CACHE[key]

    in_maps = []
    for stream in range(2):
        for b_idx in range(NBATCH):
            for half in range(2):
                in_maps.append(_prep_core_inputs(stream, b_idx, half, inputs))

    res = run_bass_kernel_spmd(nc, in_maps, list(range(8)))
    outs = [r["out"] for r in res.results]

    g_out = np.stack([outs[0] + outs[1], outs[2] + outs[3]])
    r_out = np.stack([outs[4] + outs[5], outs[6] + outs[7]])
    return (np.asarray(g_out, np.float32), np.asarray(r_out, np.float32))


# revision 25
# speedup vs baseline: 3.2041x; 1.1333x over previous
"""CoSSM (bidirectional Mamba-style SSM) Trainium2 Bass kernel.

Sharding over 8 cores: (stream g/r) x (batch 0/1) x (d_inner half 0/1).
Each core, for its (stream, batch):
  - in_proj x (all 1536 ch, channel-permuted so own half = tiles 0..5),
    recomputed per direction, fused with the causal/anticausal depthwise
    conv (diagonal PE matmuls) + native Silu, chunked over time with a
    4-column halo carried between chunks (no DRAM round trip for x)
  - xproj (full-d contraction) -> dt/B/C rows; dtproj + native Softplus
  - fwd + bwd selective scans (16 states, DVE tensor_tensor_scan);
    bwd consumes u/dl/B/C through time-reversed DMA reads (no flips)
  - D residual, bidirectional average (0.5 folded into out_proj w),
    silu(z) gate, partial out_proj -> (2048, 768)
Host sums the two partial outputs per (stream, batch).

Weights are SBUF-resident (loaded once, bf16); activations bf16 where
cheap, f32 on the delta/scan-state path.

Self-contained: hardcodes shapes. Inputs use the reference setup_inputs names.
"""
import numpy as np
import ml_dtypes

import concourse.bass as bass
import concourse.bacc as bacc
import concourse.tile as tile
import concourse.mybir as mybir
from concourse.bass_utils import run_bass_kernel_spmd

F32 = mybir.dt.float32
BF16 = mybir.dt.bfloat16
AF = mybir.ActivationFunctionType
OP = mybir.AluOpType

D_MODEL = 768
D_STATE = 16
D_CONV = 4
D_INNER = 1536
DT_RANK = 48
NBATCH = 2
L = 2048
HALF = 768
NT_FULL = 12      # 128-tiles over d_inner
NT_HALF = 6       # 128-tiles over own half
NK = 6            # 128-chunks over d_model contraction
TCH = 512         # time chunk for phase A
NTCH = L // TCH
SEG = 256         # time segment for the scan + epilogue
NSEG = L // SEG
HALO = 4

_PROGRAM_CACHE = {}


def _ap(t, offset, ap):
    return bass.AP(tensor=t.tensor, offset=offset, ap=[list(a) for a in ap])


def _bc6(view2d):
    """[128, T] AP -> [128, 6, T] read view (0-stride middle dim)."""
    a = view2d
    return bass.AP(tensor=a.tensor, offset=a.offset,
                   ap=[list(a.ap[0]), [0, NT_HALF], list(a.ap[1])])


def build_program(a_vals_f, a_vals_b):
    nc = bacc.Bacc("TRN2", target_bir_lowering=False, debug=False, num_devices=8)

    def din(name, shape, dt):
        return nc.dram_tensor(name, list(shape), dt, kind="ExternalInput").ap()

    hid_T = din("hid_T", (D_MODEL, L), BF16)
    w_in_x_T = din("w_in_x_T", (D_MODEL, D_INNER), BF16)
    w_zo_T = din("w_zo_T", (D_MODEL, 2 * D_MODEL), BF16)  # [w_in_z | 0.5*w_out]
    cdiag = {"f": din("cdiag_f", (NT_FULL * D_CONV, 128, 128), BF16),
             "b": din("cdiag_b", (NT_FULL * D_CONV, 128, 128), BF16)}
    cbias = {"f": din("cbias_f", (D_INNER,), F32),
             "b": din("cbias_b", (D_INNER,), F32)}
    w_x_T = {"f": din("w_x_T_f", (D_INNER, 80), BF16),
             "b": din("w_x_T_b", (D_INNER, 80), BF16)}
    w_dt_T = {"f": din("w_dt_T_f", (DT_RANK, HALF), F32),
              "b": din("w_dt_T_b", (DT_RANK, HALF), F32)}
    dt_bias = {"f": din("dt_bias_f", (HALF,), F32),
               "b": din("dt_bias_b", (HALF,), F32)}
    d_res = {"f": din("d_f", (HALF,), F32), "b": din("d_b", (HALF,), F32)}

    out = nc.dram_tensor("out", [L, D_MODEL], F32, kind="ExternalOutput").ap()

    u_sp = {d: nc.dram_tensor(f"u_sp_{d}", [NT_HALF, 128, L], BF16).ap()
            for d in "fb"}
    dl_sp = {d: nc.dram_tensor(f"dl_sp_{d}", [NT_HALF, 128, L], F32).ap()
             for d in "fb"}
    r_sp = {d: nc.dram_tensor(f"r_sp_{d}", [NT_HALF, 128, L], F32).ap()
            for d in "fb"}
    bc_dram = {d: nc.dram_tensor(f"bc_{d}", [2 * D_STATE, L], BF16).ap()
               for d in "fb"}

    a_vals = {"f": a_vals_f, "b": a_vals_b}

    import contextlib
    with tile.TileContext(nc) as tc, contextlib.ExitStack() as ctx:
        WPOOL = ctx.enter_context(tc.tile_pool(name="wsmall", bufs=1))
        AP_ = ctx.enter_context(tc.tile_pool(name="phaseA", bufs=1))
        SP = ctx.enter_context(tc.tile_pool(name="work", bufs=2))
        SC = ctx.enter_context(tc.tile_pool(name="scan1", bufs=1))
        SC2 = ctx.enter_context(tc.tile_pool(name="scan2", bufs=2))
        DP = ctx.enter_context(tc.tile_pool(name="phaseD", bufs=1))
        PS = ctx.enter_context(tc.tile_pool(name="psum", bufs=2, space="PSUM"))
        PS1 = ctx.enter_context(tc.tile_pool(name="psum1", bufs=1, space="PSUM"))
        PER = ctx.enter_context(tc.tile_pool(name="persist", bufs=1))

        # ---- resident weights (one DMA each) ----
        t_wbig = WPOOL.tile([128, NK, D_INNER], BF16, tag="wbig")
        nc.sync.dma_start(
            out=t_wbig,
            in_=_ap(w_in_x_T, 0,
                    [[D_INNER, 128], [128 * D_INNER, NK], [1, D_INNER]]))

        def load_cols(src, n, tagn):
            t = WPOOL.tile([128, n], F32, tag=tagn)
            nc.sync.dma_start(out=t, in_=_ap(src, 0, [[1, 128], [128, n]]))
            return t

        t_cbias = {d: load_cols(cbias[d], NT_FULL, f"cb{d}") for d in "fb"}
        t_dtb = {d: load_cols(dt_bias[d], NT_HALF, f"db{d}") for d in "fb"}
        t_dcol = {d: load_cols(d_res[d], NT_HALF, f"dd{d}") for d in "fb"}
        t_wx = {}
        for d in "fb":
            t = WPOOL.tile([128, NT_FULL, 80], BF16, tag=f"wx{d}")
            nc.sync.dma_start(
                out=t, in_=_ap(w_x_T[d], 0,
                               [[80, 128], [128 * 80, NT_FULL], [1, 80]]))
            t_wx[d] = t
        t_wdt = {}
        for d in "fb":
            t = WPOOL.tile([128, HALF], F32, tag=f"wdt{d}")
            nc.sync.dma_start(out=t[0:DT_RANK, :], in_=w_dt_T[d])
            t_wdt[d] = t

        # conv diag tiles: loaded per direction into one aliased slot
        def load_cdiag(d):
            t = WPOOL.tile([128, NT_FULL * D_CONV, 128], BF16, tag="cdiag")
            nc.sync.dma_start(
                out=t, in_=_ap(cdiag[d], 0,
                               [[128, 128], [128 * 128, NT_FULL * D_CONV],
                                [1, 128]]))
            return t

        t_xdbl = PER.tile([128, L], F32, tag="xdbl")
        t_xw = PER.tile([128, NT_FULL, TCH + HALO], BF16, tag="xw")
        carry = {d: PER.tile([128, D_STATE, NT_HALF], F32, tag=f"carry{d}",
                             name=f"carry_{d}")
                 for d in "fb"}

        # ---- phase A: fused in_proj + conv + silu + xproj + dt ----
        def emit_phaseA_chunk(d, tci, t_cd):
            t0 = tci * TCH
            first = (tci == 0) if d == "f" else (tci == NTCH - 1)
            th = AP_.tile([128, NK, TCH], BF16, tag="hidw", bufs=2)
            nc.sync.dma_start(
                out=th, in_=_ap(hid_T, t0, [[L, 128], [128 * L, NK], [1, TCH]]))
            # halo carry in xw: fwd cols 0:4 <- prev cols 512:516 (or zero);
            # bwd cols 512:516 <- prev cols 0:4 (or zero)
            if d == "f":
                if first:
                    nc.vector.memset(t_xw[:, :, 0:HALO].bitcast(F32), 0.0)
                else:
                    nc.vector.tensor_copy(t_xw[:, :, 0:HALO],
                                          t_xw[:, :, TCH:TCH + HALO])
            else:
                if first:
                    nc.vector.memset(t_xw[:, :, TCH:].bitcast(F32), 0.0)
                else:
                    nc.vector.tensor_copy(t_xw[:, :, TCH:TCH + HALO],
                                          t_xw[:, :, 0:HALO])
            xcol = HALO if d == "f" else 0

            t_useg = AP_.tile([128, NT_HALF, TCH], BF16, tag="useg")
            t_dlseg = AP_.tile([128, NT_HALF, TCH], F32, tag="dlseg")
            px = PS1.tile([128, TCH], F32, tag="xproj")
            for i in range(NT_FULL):
                ip = PS.tile([128, TCH], F32, tag="mm512")
                for k in range(NK):
                    nc.tensor.matmul(ip[:], t_wbig[:, k, i * 128:(i + 1) * 128],
                                     th[:, k, :],
                                     start=(k == 0), stop=(k == NK - 1))
                nc.scalar.copy(t_xw[:, i, xcol:xcol + TCH], ip[:])
                cp = PS.tile([128, TCH], F32, tag="mm512")
                for k in range(D_CONV):
                    off = 1 + k if d == "f" else 3 - k
                    nc.tensor.matmul(cp[:], t_cd[:, i * D_CONV + k, :],
                                     t_xw[:, i, off:off + TCH],
                                     start=(k == 0), stop=(k == D_CONV - 1))
                if i < NT_HALF:
                    u_i = t_useg[:, i, :]
                else:
                    u_hi = SP.tile([128, TCH], BF16, tag="u_hi")
                    u_i = u_hi[:]
                nc.scalar.activation(u_i, cp[:], AF.Silu,
                                     bias=t_cbias[d][:, i:i + 1], scale=1.0)
                nc.tensor.matmul(px[0:80, :], t_wx[d][:, i, :], u_i,
                                 start=(i == 0), stop=(i == NT_FULL - 1))
            nc.sync.dma_start(
                out=_ap(u_sp[d], t0, [[L, 128], [128 * L, NT_HALF], [1, TCH]]),
                in_=t_useg[:])
            nc.scalar.copy(t_xdbl[0:80, t0:t0 + TCH], px[0:80, :])
            for m in range(NT_HALF):
                dp = PS.tile([128, TCH], F32, tag="mm512")
                nc.tensor.matmul(dp[:],
                                 t_wdt[d][0:DT_RANK, m * 128:(m + 1) * 128],
                                 t_xdbl[0:DT_RANK, t0:t0 + TCH],
                                 start=True, stop=True)
                e1 = SP.tile([128, TCH], F32, tag="e1")
                nc.scalar.activation(e1[:], dp[:], AF.Exp,
                                     bias=t_dtb[d][:, m:m + 1], scale=1.0)
                nc.scalar.activation(t_dlseg[:, m, :], e1[:], AF.Ln,
                                     bias=1.0, scale=1.0)
            nc.sync.dma_start(
                out=_ap(dl_sp[d], t0, [[L, 128], [128 * L, NT_HALF], [1, TCH]]),
                in_=t_dlseg[:])

        # ---- scan ----
        def emit_scan_seg(d, s):
            # Scan seg s covers natural times [base, base+SEG). For d == "b"
            # all data stays in natural time order; only the scan instruction
            # operands use reversed views so the recurrence runs backward.
            rev = (d == "b")
            base = (L - (s + 1) * SEG) if rev else s * SEG
            u_seg = SC.tile([128, NT_HALF, SEG], BF16, tag="u_seg", bufs=2)
            nc.sync.dma_start(
                out=u_seg,
                in_=_ap(u_sp[d], base,
                        [[L, 128], [128 * L, NT_HALF], [1, SEG]]))
            dl_seg = SC.tile([128, NT_HALF, SEG], F32, tag="dl_seg")
            nc.sync.dma_start(
                out=dl_seg,
                in_=_ap(dl_sp[d], base,
                        [[L, 128], [128 * L, NT_HALF], [1, SEG]]))
            bc = SC.tile([128, 2 * D_STATE, SEG], BF16, tag="bc")
            nc.sync.dma_start(
                out=bc,
                in_=_ap(bc_dram[d], base,
                        [[0, 128], [L, 2 * D_STATE], [1, SEG]]))
            ud_seg = SC.tile([128, NT_HALF, SEG], BF16, tag="ud_seg")
            nc.vector.tensor_tensor(ud_seg[:], dl_seg[:], u_seg[:], OP.mult)
            y_seg = SC.tile([128, NT_HALF, SEG], F32, tag="y_seg")
            for j in range(D_STATE):
                a_j = SC2.tile([128, NT_HALF, SEG], BF16, tag="a_j")
                nc.scalar.activation(a_j[:], dl_seg[:], AF.Exp, bias=0.0,
                                     scale=float(a_vals[d][j]))
                b_j = SC2.tile([128, NT_HALF, SEG], BF16, tag="b_j")
                nc.vector.tensor_tensor(b_j[:], ud_seg[:], _bc6(bc[:, j, :]),
                                        OP.mult)
                h_j = SC2.tile([128, NT_HALF, SEG], BF16, tag="h_j")

                def sop(t, i):
                    v = t[:, i, :]
                    if not rev:
                        return v
                    return bass.AP(tensor=v.tensor, offset=v.offset + SEG - 1,
                                   ap=[list(v.ap[0]), [-1, SEG]])

                for i in range(NT_HALF):
                    init = 0.0 if s == 0 else carry[d][:, j, i:i + 1]
                    nc.vector.tensor_tensor_scan(
                        sop(h_j, i), sop(a_j, i), sop(b_j, i), init,
                        OP.mult, OP.add)
                if s < NSEG - 1:
                    # scan-order last element: natural col SEG-1 (fwd), 0 (bwd)
                    nc.vector.tensor_copy(
                        carry[d][:, j, :],
                        bass.AP(tensor=h_j.tensor,
                                offset=h_j[:].offset +
                                (0 if rev else SEG - 1),
                                ap=[list(h_j[:].ap[0]), [SEG, NT_HALF]]))
                nc.vector.tensor_tensor(h_j[:], h_j[:],
                                        _bc6(bc[:, D_STATE + j, :]), OP.mult)
                if j == 0:
                    nc.gpsimd.tensor_copy(out=y_seg[:], in_=h_j[:])
                elif j % 4 == 1:
                    nc.vector.tensor_tensor(y_seg[:], y_seg[:], h_j[:], OP.add)
                else:
                    nc.gpsimd.tensor_tensor(y_seg[:], y_seg[:], h_j[:], OP.add)
            r_seg = SC.tile([128, NT_HALF, SEG], F32, tag="r_seg")
            for i in range(NT_HALF):
                nc.vector.scalar_tensor_tensor(
                    r_seg[:, i, :], u_seg[:, i, :],
                    t_dcol[d][:, i:i + 1], y_seg[:, i, :], OP.mult, OP.add)
            nc.sync.dma_start(
                out=_ap(r_sp[d], base,
                        [[L, 128], [128 * L, NT_HALF], [1, SEG]]),
                in_=r_seg[:])

        # ---- phase D: z-gate + combine + out_proj ----
        def emit_D_seg(s):
            t0 = s * SEG
            th = AP_.tile([128, NK, SEG], BF16, tag="hidw", bufs=2)
            nc.sync.dma_start(
                out=th, in_=_ap(hid_T, t0, [[L, 128], [128 * L, NK], [1, SEG]]))
            gate = DP.tile([128, NT_HALF, SEG], BF16, tag="gate")
            for m in range(NT_HALF):
                zp = PS.tile([128, SEG], F32, tag="mm512")
                for k in range(NK):
                    nc.tensor.matmul(zp[:], t_wbig[:, k, m * 128:(m + 1) * 128],
                                     th[:, k, :],
                                     start=(k == 0), stop=(k == NK - 1))
                nc.scalar.activation(gate[:, m, :], zp[:], AF.Silu, bias=0.0,
                                     scale=1.0)
            rf = DP.tile([128, NT_HALF, SEG], F32, tag="rf")
            nc.sync.dma_start(
                out=rf, in_=_ap(r_sp["f"], t0,
                                [[L, 128], [128 * L, NT_HALF], [1, SEG]]))
            rb = DP.tile([128, NT_HALF, SEG], F32, tag="rb")
            nc.sync.dma_start(
                out=rb, in_=_ap(r_sp["b"], t0,
                                [[L, 128], [128 * L, NT_HALF], [1, SEG]]))
            nc.vector.tensor_tensor(rf[:], rf[:], rb[:], OP.add)
            yg = DP.tile([128, NT_HALF, SEG], BF16, tag="yg")
            nc.vector.tensor_tensor(yg[:], rf[:], gate[:], OP.mult)
            for tcc in range(SEG // 128):
                oseg = SP.tile([128, D_MODEL], F32, tag="oseg")
                for nh in range(2):
                    po = PS.tile([128, 384], F32, tag="oproj")
                    for i in range(NT_HALF):
                        nc.tensor.matmul(
                            po[:], yg[:, i, tcc * 128:(tcc + 1) * 128],
                            t_wbig[:, i, D_MODEL + nh * 384:
                                   D_MODEL + (nh + 1) * 384],
                            start=(i == 0), stop=(i == NT_HALF - 1))
                    nc.scalar.copy(oseg[:, nh * 384:(nh + 1) * 384], po[:])
                nc.sync.dma_start(
                    out=out[t0 + tcc * 128:t0 + (tcc + 1) * 128, :],
                    in_=oseg[:])

        # ---- sequencing: overlap scan(f) with A(b), scan(b) with D ----
        t_cd = load_cdiag("f")
        for tci in range(NTCH):
            emit_phaseA_chunk("f", tci, t_cd)
        nc.gpsimd.dma_start(out=bc_dram["f"],
                            in_=t_xdbl[DT_RANK:DT_RANK + 2 * D_STATE, :])
        t_cd = load_cdiag("b")
        for s in range(NSEG):
            emit_scan_seg("f", s)
            if s % 2 == 1:
                emit_phaseA_chunk("b", NTCH - 1 - s // 2, t_cd)
        nc.gpsimd.dma_start(out=bc_dram["b"],
                            in_=t_xdbl[DT_RANK:DT_RANK + 2 * D_STATE, :])
        # reload wbig slot with [w_in_z | 0.5*w_out] for phase D
        nc.sync.dma_start(
            out=t_wbig,
            in_=_ap(w_zo_T, 0,
                    [[2 * D_MODEL, 128], [128 * 2 * D_MODEL, NK],
                     [1, 2 * D_MODEL]]))
        for s in range(NSEG):
            emit_scan_seg("b", s)
            emit_D_seg(NSEG - 1 - s)

    nc.compile()
    return nc


def _diags(w):  # (1536, 4) -> (48, 128, 128) diag tiles
    o = np.zeros((NT_FULL * D_CONV, 128, 128), np.float32)
    for i in range(NT_FULL):
        for k in range(D_CONV):
            np.fill_diagonal(o[i * D_CONV + k], w[i * 128:(i + 1) * 128, k])
    return o


def _bf(a):
    return np.ascontiguousarray(np.asarray(a, np.float32).astype(
        ml_dtypes.bfloat16))


def _f32(a):
    return np.ascontiguousarray(np.asarray(a), dtype=np.float32)


def _prep_core_inputs(stream, b_idx, half, inp):
    p = "g" if stream == 0 else "r"
    h0, h1 = half * HALF, (half + 1) * HALF
    perm = np.r_[h0:h1, 0:h0, h1:D_INNER]  # own half first

    hs = np.asarray(inp[f"{p}_hidden_states"])[b_idx]
    w_in = np.asarray(inp[f"{p}_in_proj_w"])
    w_zo = np.concatenate(
        [np.asarray(w_in[D_INNER + h0:D_INNER + h1, :]).T,
         0.5 * np.asarray(inp[f"{p}_out_w"])[:, h0:h1].T], axis=1)
    m = {
        "hid_T": _bf(hs.T),
        "w_in_x_T": _bf(w_in[:D_INNER, :][perm].T),
        "w_zo_T": _bf(w_zo),
        "cdiag_f": _bf(_diags(np.asarray(inp[f"{p}_conv_w"])[:, 0, :][perm])),
        "cdiag_b": _bf(_diags(np.asarray(inp[f"{p}_conv_w_bwd"])[:, 0, :][perm])),
        "cbias_f": _f32(np.asarray(inp[f"{p}_conv_bias"])[perm]),
        "cbias_b": _f32(np.asarray(inp[f"{p}_conv_bias_bwd"])[perm]),
        "w_x_T_f": _bf(np.asarray(inp[f"{p}_xproj_w"])[:, perm].T),
        "w_x_T_b": _bf(np.asarray(inp[f"{p}_xproj_w_bwd"])[:, perm].T),
        "w_dt_T_f": _f32(np.asarray(inp[f"{p}_dtproj_w"])[h0:h1, :].T),
        "w_dt_T_b": _f32(np.asarray(inp[f"{p}_dtproj_w_bwd"])[h0:h1, :].T),
        "dt_bias_f": _f32(np.asarray(inp[f"{p}_dtproj_bias"])[h0:h1]),
        "dt_bias_b": _f32(np.asarray(inp[f"{p}_dtproj_bias_bwd"])[h0:h1]),
        "d_f": _f32(np.asarray(inp[f"{p}_D"])[h0:h1]),
        "d_b": _f32(np.asarray(inp[f"{p}_D_bwd"])[h0:h1]),
    }
    return m


def kernel(**inputs):
    A_log = np.asarray(inputs["A_log"])
    A_log_b = np.asarray(inputs["A_log_bwd"])
    assert np.allclose(A_log, A_log[0:1, :]), "A_log must be d-independent"
    assert np.allclose(A_log_b, A_log_b[0:1, :]), "A_log_bwd must be d-independent"
    A_f = -np.exp(A_log[0].astype(np.float64))
    A_b = -np.exp(A_log_b[0].astype(np.float64))

    key = (tuple(np.round(A_f, 10)), tuple(np.round(A_b, 10)))
    if key not in _PROGRAM_CACHE:
        _PROGRAM_CACHE[key] = build_program(list(A_f), list(A_b))
    nc = _PROGRAM_CACHE[key]

    in_maps = []
    for stream in range(2):
        for b_idx in range(NBATCH):
            for half in range(2):
                in_maps.append(_prep_core_inputs(stream, b_idx, half, inputs))

    res = run_bass_kernel_spmd(nc, in_maps, list(range(8)))
    outs = [r["out"] for r in res.results]

    g_out = np.stack([outs[0] + outs[1], outs[2] + outs[3]])
    r_out = np.stack([outs[4] + outs[5], outs[6] + outs[7]])
    return (np.asarray(g_out, np.float32), np.asarray(r_out, np.float32))
